# revision 3
# baseline (speedup 1.0000x reference)
"""Trainium2 Bass kernel for nn_DilatedAttention — fp16 pipeline.

Math per image (one core):
  pooled[c] = mean_hw(x)                              (64,)
  lf = tanh(BN(pooled @ conv_w.T))                    (72,) = (G=8, k2=9)
  low[c,h,w] = sum_t lf[g(c),t] * x[c, h+di, w+dj]    3x3 reflect-pad conv
  out = A[c]*low + B[c]*x + const[c]
    A = lamb_l*(1+inside_all), B = 1+lamb_h, const = -inside_all*lamb_l*pooled

Pipeline (per core):
  1. SWDGE cast-DMA loads x (f32 HBM) directly into SBUF as fp16 at the
     HBM-read roofline [128 partitions = 2 half-images x 64ch, 130 rows
     x 256 cols with vertical reflect halos].  While chunks land, the
     column-shifted copy xs (x shifted left by 1; last col pre-reflected)
     is built with pooling fused in: ScalarE Copy+accum_out chunks and
     DVE tensor_scalar+accum_out chunks.  xs gives every horizontal tap
     a 4B-aligned contiguous view.
  2. Short serial chain after the last chunk: pstat -> stat ->
     (lf matmul | cvec matmul) -> tanh -> wmat -> 9 fp16 diag matrices;
     dummy warm-up matmuls gated on a late chunk hold the PE HAM clock
     at 2.4 GHz through the chain.
  3. Main loop over 16 units x 8 rows: 12 units on PE (9 diag matmuls
     per 2-row subtile, fp16 moving at 216 ns/MM, f32 PSUM; ScalarE
     eviction adds const into f32 staging; DVE batches the reflect
     left-edge fixes; HWDGE f32 stores on the SP+ACT rings), 4 units on
     DVE (tensor_scalar plane + tensor_tensor add per tap in fp16
     2x/4x modes, center tap + const via the two-scalar tensor_scalar
     init; fp16 staging, SWDGE cast-DMA stores back to f32).
"""

import os
import sys

import numpy as np

for _p in ("/opt/trn_rl_repo",):
    if _p not in sys.path:
        sys.path.insert(0, _p)

import concourse.bass as bass
import concourse.bacc as bacc
import concourse.mybir as mybir
import concourse.tile as tile
from concourse.bass_utils import run_bass_kernel_spmd

F32 = mybir.dt.float32
F16 = mybir.dt.float16
AF = mybir.ActivationFunctionType
ALU = mybir.AluOpType

C, H, W = 64, 256, 256
NCORES = 8
K2 = 9

# chunk boundaries in layout-row space [1, 129); last chunk small so the
# pooling tail off the final DMA is short
CHUNKS = [(1, 17), (17, 33), (33, 49), (49, 65), (65, 81), (81, 97),
          (97, 113), (113, 124), (124, 129)]
# chunks pooled+shifted on ScalarE (fused via accum_out); rest on DVE
SCALAR_CHUNKS = {0, 1, 2}

# unit u covers layout rows [1+8u, 9+8u); which units go to the PE path
PE_UNITS = set(range(16)) - {3, 7, 11, 15}

LAST_RESULT = {}


def _install_ntff_hook():
    """Register the axon NTFF profile hook (the image's antenv lacks
    axon_hooks; build it from trn_agent_boot's ctypes shim)."""
    import types

    try:
        from antenv.axon_hooks import get_axon_ntff_profile_hook  # noqa: F401
        return
    except ImportError:
        pass
    mod = types.ModuleType("antenv.axon_hooks")
    _h = [None]
    mod.set_axon_ntff_profile_hook = lambda hook: _h.__setitem__(0, hook)
    mod.get_axon_ntff_profile_hook = lambda: _h[0]
    sys.modules["antenv.axon_hooks"] = mod
    import antenv

    antenv.axon_hooks = mod
    try:
        from trn_agent_boot.trn_boot import _ntff_profile_via_ctypes

        mod.set_axon_ntff_profile_hook(
            _ntff_profile_via_ctypes("/opt/axon/libaxon_pjrt.so")
        )
    except Exception as e:
        print("ntff hook install failed:", e)


def _build_program():
    nc = bacc.Bacc("TRN2", target_bir_lowering=False, debug=False)

    x_d = nc.declare_dram_parameter("x", [C, H, W], F32, isOutput=False)
    out_d = nc.declare_dram_parameter("out", [C, H, W], F32, isOutput=True)
    cwT128_d = nc.declare_dram_parameter("cwT128", [128, 72], F32, isOutput=False)
    mcl_d = nc.declare_dram_parameter("mcl", [128, 128], F32, isOutput=False)
    bns_d = nc.declare_dram_parameter("bns", [72, 1], F32, isOutput=False)
    bnb_d = nc.declare_dram_parameter("bnb", [72, 1], F32, isOutput=False)
    g72_d = nc.declare_dram_parameter("g72", [72, 128], F32, isOutput=False)
    mask9_d = nc.declare_dram_parameter("mask9", [72, K2], F32, isOutput=False)
    i128_d = nc.declare_dram_parameter("i128", [128, 128], F16, isOutput=False)
    avec_d = nc.declare_dram_parameter("avec", [128, 1], F32, isOutput=False)
    bvec_d = nc.declare_dram_parameter("bvec", [128, 1], F32, isOutput=False)

    with tile.TileContext(nc) as tc:
        with (
            tc.tile_pool(name="xbuf", bufs=1) as xp,
            tc.tile_pool(name="consts", bufs=1) as cp,
            tc.tile_pool(name="diag", bufs=1) as dp,
            tc.tile_pool(name="psum", bufs=5, space=bass.MemorySpace.PSUM) as pp,
            tc.tile_pool(name="stage", bufs=4) as sp,
            tc.tile_pool(name="spsum", bufs=1, space=bass.MemorySpace.PSUM) as pps,
        ):
            # ---- small constants ----
            cwT128 = cp.tile([128, 72], F32, tag="cwT128")
            mcl = cp.tile([128, 128], F32, tag="mcl")
            bns = cp.tile([72, 1], F32, tag="bns")
            bnb = cp.tile([72, 1], F32, tag="bnb")
            g72 = cp.tile([72, 128], F32, tag="g72")
            mask9 = cp.tile([72, K2], F32, tag="mask9")
            i128 = cp.tile([128, 128], F16, tag="i128")
            avec = cp.tile([128, 1], F32, tag="avec")
            bvec = cp.tile([128, 1], F32, tag="bvec")
            for t, d in (
                (cwT128, cwT128_d), (mcl, mcl_d), (bns, bns_d), (bnb, bnb_d),
                (g72, g72_d), (mask9, mask9_d), (i128, i128_d),
                (avec, avec_d), (bvec, bvec_d),
            ):
                nc.sync.dma_start(out=t[:], in_=d[:])

            # preload the tanh table set early (off the critical path)
            warm = cp.tile([128, 1], F32, tag="warm")
            nc.vector.memset(warm[:], 0.0)
            nc.scalar.activation(warm[:], warm[:], AF.Tanh)

            # ---- load x into SBUF as fp16 (SWDGE cast-DMA) ----
            # Top half p<64: layout row r holds HBM row r-1; bottom half
            # p>=64: layout row r holds HBM row 127+r.  Rows 0/129 are
            # vertical reflect halos.
            x16 = xp.tile([128, 130, 256], F16)
            xs = xp.tile([128, 130, 256], F16)
            for a, b in CHUNKS:
                nc.gpsimd.dma_start(out=x16[0:64, a:b, :], in_=x_d[:, a - 1:b - 1, :])
                nc.gpsimd.dma_start(out=x16[64:128, a:b, :],
                                    in_=x_d[:, 127 + a:127 + b, :])
            # halos: top row 129 = HBM 128; top row 0 = reflect(HBM -1) = HBM 1
            # bottom row 0 = HBM 127; bottom row 129 = reflect(HBM 256) = HBM 254
            nc.gpsimd.dma_start(out=x16[0:64, 129:130, :], in_=x_d[:, 128:129, :])
            nc.gpsimd.dma_start(out=x16[0:64, 0:1, :], in_=x_d[:, 1:2, :])
            nc.gpsimd.dma_start(out=x16[64:128, 0:1, :], in_=x_d[:, 127:128, :])
            nc.gpsimd.dma_start(out=x16[64:128, 129:130, :], in_=x_d[:, 254:255, :])

            # ---- per-chunk pooling + shifted-copy build ----
            # xs[p, r, w] = x16[p, r, w+1] (w<255); xs[:, :, 255] = x16[:, :, 254]
            # (so the right tap needs no edge fix: x[256] reflects to x[254]).
            # ScalarE chunks: Copy with accum_out pools cols 1..255 for free.
            # DVE chunks: 2x_2P copy + tensor_reduce over cols 1..255.
            # Column 0 of all image rows is summed separately at the end.
            NST = len(CHUNKS) + 2
            pstat = cp.tile([128, NST], F32, tag="pstat")
            for k, (a, b) in enumerate(CHUNKS):
                if k in SCALAR_CHUNKS:
                    nc.scalar.activation(
                        xs[:, a:b, 0:255], x16[:, a:b, 1:256], AF.Copy,
                        accum_out=pstat[:, k:k + 1],
                    )
                else:
                    nc.vector.tensor_scalar(
                        xs[:, a:b, 0:255], x16[:, a:b, 1:256], 1.0, 0.0,
                        ALU.mult, ALU.add, accum_out=pstat[:, k:k + 1],
                    )
            # halo rows' shifted copies (not pooled)
            nc.vector.tensor_copy(xs[:, 0:1, 0:255], x16[:, 0:1, 1:256])
            nc.vector.tensor_copy(xs[:, 129:130, 0:255], x16[:, 129:130, 1:256])
            # xs last column = x16 col 254 (reflect of col 256)
            nc.vector.tensor_copy(xs[:, :, 255:256], x16[:, :, 254:255])
            # column-0 sum over image rows
            nc.vector.tensor_reduce(
                out=pstat[:, NST - 2:NST - 1], in_=x16[:, 1:129, 0:1],
                axis=mybir.AxisListType.XY, op=ALU.add,
            )
            nc.vector.memset(pstat[:, NST - 1:NST], 0.0)

            # PE warm-up: dummy matmuls gated on the second-to-last load
            # chunk keep the HAM clock-gate warm through the filter
            # derivation chain (PE is FIFO, so these must be issued BEFORE
            # the lf/cvec matmuls in program order)
            ww = pps.tile([128, 2, 256], F32, tag="ww")
            for i in range(16):
                nc.tensor.matmul(ww[:], i128[:],
                                 x16[:, 113 + (i % 9):115 + (i % 9), :],
                                 start=(i == 0), stop=(i == 15))

            # ---- filter derivation (short serial chain) ----
            stat = cp.tile([128, 1], F32, tag="stat")
            nc.vector.tensor_reduce(
                out=stat[:], in_=pstat[:], axis=mybir.AxisListType.X, op=ALU.add
            )
            # lf_ps[72] = sum_q cwT128[q,k]*stat[q]  (pool-normalize + half-sum
            # folded into cwT128); cvec_ps[p] = CL[p]*pooled[p] likewise
            lf_ps = pps.tile([72, 1], F32, tag="lf_ps")
            nc.tensor.matmul(lf_ps[:], cwT128[:], stat[:])
            cvec_ps = pps.tile([128, 1], F32, tag="cvec_ps")
            nc.tensor.matmul(cvec_ps[:], mcl[:], stat[:])
            lf = cp.tile([72, 1], F32, tag="lf")
            nc.scalar.activation(lf[:], lf_ps[:], AF.Tanh, bias=bnb[:], scale=bns[:])
            cvec = cp.tile([128, 1], F32, tag="cvec")
            nc.scalar.copy(cvec[:], cvec_ps[:])

            # W0[p,t] = lf[g(p)*9+t]:  lfmat = mask9 * lf ; W0 = g72.T @ lfmat
            lfmat = cp.tile([72, K2], F32, tag="lfmat")
            nc.vector.tensor_scalar_mul(lfmat[:], mask9[:], lf[:])
            w_ps = pps.tile([128, K2], F32, tag="lf_ps")
            nc.tensor.matmul(w_ps[:], g72[:], lfmat[:])
            # wmat = A * W0 ; center tap += B  (folds B*x into the conv)
            wmat = cp.tile([128, K2], F32, tag="wmat")
            nc.scalar.activation(wmat[:], w_ps[:], AF.Copy, scale=avec[:])
            nc.vector.tensor_scalar_add(wmat[:, 4:5], wmat[:, 4:5], bvec[:])
            w16 = cp.tile([128, K2], F16, tag="w16")
            nc.vector.tensor_copy(w16[:], wmat[:])

            # fp16 diagonal stationary matrices D_t = diag(wmat[:,t]);
            # split across DVE and ScalarE to shorten the serial chain
            diags = []
            for t in range(K2):
                d_t = dp.tile([128, 128], F16, tag=f"d{t}", name=f"d{t}")
                if t % 2 == 0:
                    nc.vector.tensor_scalar_mul(d_t[:], i128[:], wmat[:, t:t + 1])
                else:
                    nc.scalar.activation(d_t[:], i128[:], AF.Identity,
                                         scale=wmat[:, t:t + 1])
                diags.append(d_t)

            # ---- main loop: 16 units x 8 layout rows ----
            for u in range(16):
                base = 1 + 8 * u
                if u in PE_UNITS:
                    # 9 diag matmuls per 2-row subtile (f32 PSUM), ScalarE
                    # eviction adds const; left-edge fixes batched on DVE
                    st = sp.tile([128, 8, 256], F32, tag="st32", name="st32",
                                 bufs=3)
                    for s in range(4):
                        rr = base + 2 * s
                        ps = pp.tile([128, 2, 256], F32, tag="ps", name="ps")
                        view = st[:, 2 * s:2 * s + 2, :]
                        mm = 0
                        for dr in (-1, 0, 1):
                            rows = slice(rr + dr, rr + dr + 2)
                            tC = (dr + 1) * 3 + 1   # dw=0
                            tR = (dr + 1) * 3 + 2   # dw=+1
                            tL = (dr + 1) * 3       # dw=-1
                            nc.tensor.matmul(ps[:, :, :], diags[tC][:],
                                             x16[:, rows, 0:256],
                                             start=(mm == 0), stop=False)
                            mm += 1
                            nc.tensor.matmul(ps[:, :, :], diags[tR][:],
                                             xs[:, rows, 0:256],
                                             start=False, stop=False)
                            mm += 1
                            nc.tensor.matmul(ps[:, :, 2:256], diags[tL][:],
                                             xs[:, rows, 0:254],
                                             start=False, stop=(dr == 1))
                            mm += 1
                        nc.scalar.activation(view, ps[:], AF.Identity,
                                             bias=cvec[:])
                    # left-edge fixes (cols 0/1, reflect) in 4-row groups so
                    # stores pipeline behind the evictions instead of
                    # draining in one 1 MB tail
                    for h in range(2):
                        r0 = base + 4 * h
                        for i, dr in enumerate((-1, 0, 1)):
                            rows = slice(r0 + dr, r0 + dr + 4)
                            tL = 3 * i
                            nc.vector.scalar_tensor_tensor(
                                st[:, 4 * h:4 * h + 4, 0:2],
                                x16[:, rows, 1::-1],
                                wmat[:, tL:tL + 1],
                                st[:, 4 * h:4 * h + 4, 0:2],
                                ALU.mult, ALU.add,
                            )
                        nc.sync.dma_start(
                            out=out_d[:, base - 1 + 4 * h:base + 3 + 4 * h, :],
                            in_=st[0:64, 4 * h:4 * h + 4, :])
                        nc.scalar.dma_start(
                            out=out_d[:, 127 + base + 4 * h:131 + base + 4 * h, :],
                            in_=st[64:128, 4 * h:4 * h + 4, :])
                else:
                    # DVE unit: tensor_scalar plane + tensor_tensor add per
                    # tap (fp16 2x/4x modes), init carries center tap + const
                    st = sp.tile([128, 8, 256], F16, tag="st16", name="st16",
                                 bufs=2)
                    plane = sp.tile([128, 8, 256], F16, tag="plane",
                                    name="plane", bufs=1)
                    R = slice(base, base + 8)
                    acc = st[:, 0:8, :]
                    nc.vector.tensor_scalar(acc, x16[:, R, :], wmat[:, 4:5],
                                            cvec[:], ALU.mult, ALU.add)
                    for dr in (-1, 0, 1):
                        rows = slice(base + dr, base + dr + 8)
                        tC = (dr + 1) * 3 + 1
                        tR = (dr + 1) * 3 + 2
                        tL = (dr + 1) * 3
                        if dr != 0:
                            nc.vector.tensor_scalar(
                                plane[:, :, :], x16[:, rows, :],
                                wmat[:, tC:tC + 1], None, ALU.mult)
                            nc.vector.tensor_tensor(
                                out=acc, in0=plane[:, :, :], in1=acc,
                                op=ALU.add)
                        nc.vector.tensor_scalar(
                            plane[:, :, :], xs[:, rows, :],
                            wmat[:, tR:tR + 1], None, ALU.mult)
                        nc.vector.tensor_tensor(
                            out=acc, in0=plane[:, :, :], in1=acc, op=ALU.add)
                        nc.vector.tensor_scalar(
                            plane[:, :, 0:254], xs[:, rows, 0:254],
                            wmat[:, tL:tL + 1], None, ALU.mult)
                        nc.vector.tensor_tensor(
                            out=acc[:, :, 2:256], in0=plane[:, :, 0:254],
                            in1=acc[:, :, 2:256], op=ALU.add)
                        nc.vector.scalar_tensor_tensor(
                            acc[:, :, 0:2], x16[:, rows, 1::-1],
                            w16[:, tL:tL + 1], acc[:, :, 0:2],
                            ALU.mult, ALU.add,
                        )
                    del acc
                    nc.gpsimd.dma_start(out=out_d[:, base - 1:base + 7, :],
                                        in_=st[0:64])
                    nc.gpsimd.dma_start(out=out_d[:, 127 + base:135 + base, :],
                                        in_=st[64:128])

    nc.compile()
    return nc


def _host_consts(conv_w, bn_gamma, bn_beta, bn_mean, bn_var, lamb_l, lamb_h,
                 inside_all):
    f = np.float32
    eps = 1e-5
    bns = (bn_gamma / np.sqrt(bn_var + eps)).astype(f)          # (72,)
    bnb = (bn_beta - bn_mean * bns).astype(f)
    g = np.arange(128) % 64 // 8                                 # group of p
    g72 = np.zeros((72, 128), f)
    for p in range(128):
        for k in range(72):
            if k // 9 == g[p]:
                g72[k, p] = 1.0
    mask9 = np.zeros((72, K2), f)
    mask9[np.arange(72), np.arange(72) % 9] = 1.0
    # cwT128[q, k] = conv_w[k, q%64] / 65536  (q = partition of stat)
    cwT128 = np.zeros((128, 72), f)
    for q in range(128):
        cwT128[q, :] = conv_w[:, q % 64] / 65536.0
    ia = inside_all.reshape(-1).astype(f)                        # (64,)
    ll = lamb_l.astype(f)
    lh = lamb_h.astype(f)
    a64 = (ll * (1.0 + ia)).astype(f)
    b64 = (1.0 + lh).astype(f)
    cl64 = (-ia * ll).astype(f)
    # mcl[q, p] = cl64[p%64]/65536 if q%64 == p%64 (lhsT for cvec matmul)
    mcl = np.zeros((128, 128), f)
    for q in range(128):
        for p in range(128):
            if q % 64 == p % 64:
                mcl[q, p] = cl64[p % 64] / 65536.0
    dup = lambda v: np.concatenate([v, v]).reshape(128, 1).astype(f)
    return dict(
        cwT128=cwT128,
        mcl=mcl,
        bns=bns.reshape(72, 1),
        bnb=bnb.reshape(72, 1),
        g72=g72,
        mask9=mask9,
        i128=np.eye(128, dtype=np.float16),
        avec=dup(a64),
        bvec=dup(b64),
    )


def kernel(x, conv_w, bn_gamma, bn_beta, bn_mean, bn_var, lamb_l, lamb_h,
           inside_all):
    x = np.asarray(x, np.float32)
    consts = _host_consts(
        np.asarray(conv_w, np.float32), np.asarray(bn_gamma, np.float32),
        np.asarray(bn_beta, np.float32), np.asarray(bn_mean, np.float32),
        np.asarray(bn_var, np.float32), np.asarray(lamb_l, np.float32),
        np.asarray(lamb_h, np.float32), np.asarray(inside_all, np.float32),
    )
    nc = _build_program()
    in_maps = [
        dict(x=np.ascontiguousarray(x[i]), **consts) for i in range(NCORES)
    ]
    trace = bool(os.environ.get("BASS_TRACE_KERNEL"))
    if trace:
        _install_ntff_hook()
    res = run_bass_kernel_spmd(
        nc, in_maps, core_ids=list(range(NCORES)), trace=trace
    )
    LAST_RESULT["exec_time_ns"] = res.exec_time_ns
    LAST_RESULT["raw"] = res
    return np.stack([res.results[i]["out"] for i in range(NCORES)], axis=0)
